# revision 1
# baseline (speedup 1.0000x reference)
"""HRR self-attention (causal holographic binding) on 8 Trainium2 cores.

Math (per batch b, head h, reference semantics):
    qkv = x @ w_qkv ; q,k,v heads of HD=128
    fq,fk,fv = fft(q|k|v, axis=-1)          (length-128 FFT == matmul with DFT matrix)
    kv   = cumsum(fk*fv, axis=seq)          (causal binding)
    vals = ifft(kv * conj(fq)).real
    out  = vals @ w_out

Implementation notes:
  * FFT/iFFT are 128x128 matmuls (HD == 128 == PE tile).  Real-input FFT is
    conjugate-symmetric, so the full spectrum is packed into 128 partition
    rows:  p=0 -> Re bin0, p=1 -> Re bin64, p=2..64 -> Re bins 1..63,
    p=65..127 -> Im bins 1..63.  This makes every cumsum a contiguous-
    partition tensor_tensor_scan along the free (token) axis.
  * Sharding: core c = 2*b + g handles batch b, heads 4g..4g+3.  Each core
    emits a partial out^T (its 4 heads' contribution); the host sums the
    pair of partials per batch.  No cross-core communication.
  * All matmuls run in fp16 (fp32 PSUM accumulate).  DFT matrices are
    pre-scaled by 1/16 per FFT application to keep intermediates inside
    fp16 range; the inverse matrices and the host-side final scale undo it.
  * Scans keep fp32 state on-engine; data/products/outputs are fp16.
"""

import numpy as np

B, S, D, H = 4, 4096, 1024, 8
HD = 128
NCORES = 8
HPC = H // 2            # heads per core
T = 512                 # token chunk (PSUM bank = 512 fp32)
NT = S // T
KK = D // 128           # contraction tiles for the qkv projection
FS = 16.0               # scale folded into each forward DFT matrix
SV = 16.0               # vals stored as vals/SV
SO = 16.0               # outT stored as out/SO  (host multiplies back)


def _build_consts():
    """Forward packed DFT matrices Gk|Gv|Gsn|Gs0 and inverse A1|A2.

    Packed layout (partition row p): p=0..63 -> Re bins 0..63,
    p=64 -> Re bin 64 (Nyquist), p=65..127 -> Im bins 1..63.

    Walrus requires equal partition bases for the two SBUF inputs of a
    DVE op, so the binding products are built from *pre-swapped* spectra
    (fks = Gs0.T k puts Im content in rows 0..63) and every scan reads
    two operands at the same base:
        re-scan rows [0:64):   cumsum(fk*fv [0:64] - fks*fvs [0:64])
        im-scan rows [64:128): cumsum(fk*fvs [64:] + fks*fv [64:])
    The Nyquist bin rides in row 64 of the im-scan: Gk/Gsn carry (-1)^a
    in col 64 while Gv/Gs0 zero it, so (fk*fvs)[64] = fk64*fv64 and
    (fks*fv)[64] = 0.
    """
    n = HD
    a = np.arange(n)
    cos_aj = np.cos(2 * np.pi * np.outer(a, np.arange(64)) / n)   # [a, j]
    sin_aj = np.sin(2 * np.pi * np.outer(a, np.arange(64)) / n)
    nyq = np.where(a % 2 == 0, 1.0, -1.0)              # (-1)^a

    def fwd(re_cols, col64, im_cols):
        M = np.zeros((n, n))
        M[:, :64] = re_cols
        M[:, 64] = col64
        M[:, 65:] = im_cols[:, 1:]                     # im bins 1..63
        return M

    Gk = fwd(cos_aj, nyq, -sin_aj)                     # fq uses Gk too
    Gv = fwd(cos_aj, 0.0, -sin_aj)
    Gsn = fwd(-sin_aj, nyq, cos_aj)                    # swapped, Nyquist col (for v)
    Gs0 = fwd(-sin_aj, 0.0, cos_aj)                    # swapped, zero col (for k, q)

    # inverse: vals_n = sum_p A1[p,n] P1[p] + A2[p,n] P2[p]
    cos_jn = np.cos(2 * np.pi * np.outer(np.arange(64), a) / n)   # [j, n]
    sin_jn = np.sin(2 * np.pi * np.outer(np.arange(64), a) / n)
    w = np.full(64, 2.0)
    w[0] = 1.0
    A1 = np.zeros((n, n))
    A1[:64, :] = w[:, None] * cos_jn / n
    A1[64, :] = np.where(np.arange(n) % 2 == 0, 1.0, -1.0) / n    # Nyquist (-1)^n
    A1[65:, :] = 2.0 * cos_jn[1:] / n
    A2 = np.zeros((n, n))
    A2[:64, :] = 2.0 * sin_jn / n
    A2[64, :] = 0.0
    A2[65:, :] = -2.0 * sin_jn[1:] / n

    Amul = FS ** 3 / SV
    gmat = np.concatenate(
        [Gk / FS, Gv / FS, Gsn / FS, Gs0 / FS], axis=1).astype(np.float16)  # [128, 512]
    amat = np.concatenate([A1 * Amul, A2 * Amul], axis=1).astype(np.float16)  # [128, 256]
    return gmat, amat


def _build_program():
    import concourse.bass as bass
    import concourse.bacc as bacc
    import concourse.mybir as mybir
    import concourse.tile as tile

    f16 = mybir.dt.float16
    f32 = mybir.dt.float32
    add = mybir.AluOpType.add
    sub = mybir.AluOpType.subtract

    nc = bacc.Bacc("TRN2", target_bir_lowering=False, debug=False)
    xT = nc.dram_tensor("xT", [D, S], f16, kind="ExternalInput").ap()
    wq = nc.dram_tensor("wq", [D, 3 * HPC * 128], f16, kind="ExternalInput").ap()
    wo = nc.dram_tensor("wo", [HPC * 128, D], f16, kind="ExternalInput").ap()
    gmat = nc.dram_tensor("gmat", [128, 512], f16, kind="ExternalInput").ap()
    amat = nc.dram_tensor("amat", [128, 256], f16, kind="ExternalInput").ap()
    outT = nc.dram_tensor("outT", [D, S], f16, kind="ExternalOutput").ap()

    with tile.TileContext(nc) as tc:
        with (
            tc.tile_pool(name="consts", bufs=1) as cpool,
            tc.tile_pool(name="xin", bufs=2) as xpool,
            tc.tile_pool(name="work", bufs=2) as wpool,
            tc.tile_pool(name="kvp", bufs=2) as kvpool,
            tc.tile_pool(name="psA", bufs=1, space="PSUM") as psA,
            tc.tile_pool(name="psB", bufs=1, space="PSUM") as psB,
            tc.tile_pool(name="psC", bufs=1, space="PSUM") as psC,
        ):
            wq_sb = []
            for k in range(KK):
                wqt = cpool.tile([128, 3 * HPC * 128], f16, name=f"wq{k}")
                nc.sync.dma_start(out=wqt, in_=wq[k * 128:(k + 1) * 128, :])
                wq_sb.append(wqt)
            wo_sb = []
            for h in range(HPC):
                wot = cpool.tile([128, D], f16, name=f"wo{h}")
                nc.sync.dma_start(out=wot, in_=wo[h * 128:(h + 1) * 128, :])
                wo_sb.append(wot)
            g_sb = cpool.tile([128, 512], f16, name="g_sb")
            nc.sync.dma_start(out=g_sb, in_=gmat)
            a_sb = cpool.tile([128, 256], f16, name="a_sb")
            nc.sync.dma_start(out=a_sb, in_=amat)

            kv_prev = [None] * HPC
            for t in range(NT):
                ts = slice(t * T, (t + 1) * T)
                xk = []
                for k in range(KK):
                    xkt = xpool.tile([128, T], f16, tag=f"xk{k}", name=f"x_{t}_{k}")
                    nc.sync.dma_start(out=xkt, in_=xT[k * 128:(k + 1) * 128, ts])
                    xk.append(xkt)
                vals_sb = []
                for h in range(HPC):
                    # projection q|k|v for this head
                    ps_qkv = psA.tile([128, 3 * T], f32, tag="A", name=f"psqkv_{t}_{h}")
                    for comp in range(3):
                        col0 = (h * 3 + comp) * 128
                        for k in range(KK):
                            nc.tensor.matmul(
                                ps_qkv[:, comp * T:(comp + 1) * T],
                                lhsT=wq_sb[k][:, col0:col0 + 128],
                                rhs=xk[k],
                                start=(k == 0),
                                stop=(k == KK - 1),
                            )
                    qkv_sb = wpool.tile([128, 3 * T], f16, tag="qkv", name=f"qkv_{t}_{h}")
                    nc.vector.tensor_copy(qkv_sb, ps_qkv)
                    qs = qkv_sb[:, 0:T]
                    ks = qkv_sb[:, T:2 * T]
                    vs = qkv_sb[:, 2 * T:3 * T]
                    # packed FFTs
                    ps_fkv = psB.tile([128, 4 * T], f32, tag="B", name=f"psfkv_{t}_{h}")
                    nc.tensor.matmul(ps_fkv[:, 0:T], lhsT=g_sb[:, 0:128], rhs=ks)
                    nc.tensor.matmul(ps_fkv[:, T:2 * T], lhsT=g_sb[:, 384:512], rhs=ks)
                    nc.tensor.matmul(ps_fkv[:, 2 * T:3 * T], lhsT=g_sb[:, 128:256], rhs=vs)
                    nc.tensor.matmul(ps_fkv[:, 3 * T:4 * T], lhsT=g_sb[:, 256:384], rhs=vs)
                    ps_fq2 = psA.tile([128, 2 * T], f32, tag="A", name=f"psfq_{t}_{h}")
                    nc.tensor.matmul(ps_fq2[:, 0:T], lhsT=g_sb[:, 0:128], rhs=qs)
                    nc.tensor.matmul(ps_fq2[:, T:2 * T], lhsT=g_sb[:, 384:512], rhs=qs)
                    fkv_sb = wpool.tile([128, 4 * T], f16, tag="fkv", name=f"fkv_{t}_{h}")
                    nc.scalar.copy(fkv_sb, ps_fkv)
                    fq2_sb = wpool.tile([128, 2 * T], f16, tag="fq2", name=f"fq2_{t}_{h}")
                    nc.scalar.copy(fq2_sb, ps_fq2)
                    fk_s = fkv_sb[:, 0:T]
                    fks_s = fkv_sb[:, T:2 * T]
                    fv_s = fkv_sb[:, 2 * T:3 * T]
                    fvs_s = fkv_sb[:, 3 * T:4 * T]
                    fq_s = fq2_sb[:, 0:T]
                    fqs_s = fq2_sb[:, T:2 * T]
                    # binding products: Pa=fk*fv Pb=fks*fvs Pc=fk*fvs Pd=fks*fv
                    pk = wpool.tile([128, 4 * T], f16, tag="pk", name=f"pk_{t}_{h}")
                    nc.vector.tensor_mul(pk[0:64, 0:T], fk_s[0:64, :], fv_s[0:64, :])
                    nc.vector.tensor_mul(pk[0:64, T:2 * T], fks_s[0:64, :], fvs_s[0:64, :])
                    nc.vector.tensor_mul(pk[64:128, 2 * T:3 * T], fk_s[64:128, :], fvs_s[64:128, :])
                    nc.vector.tensor_mul(pk[64:128, 3 * T:4 * T], fks_s[64:128, :], fv_s[64:128, :])
                    # causal cumsum (carry chained across chunks)
                    kvt = kvpool.tile([128, T], f16, tag=f"kv{h}", name=f"kv_{t}_{h}")
                    if t == 0:
                        init_r = init_i = 0.0
                    else:
                        p = kv_prev[h]
                        init_r = p[0:64, T - 1:T]
                        init_i = p[64:128, T - 1:T]
                    nc.vector.tensor_tensor_scan(
                        kvt[0:64, :], pk[0:64, 0:T], pk[0:64, T:2 * T], init_r, add, sub)
                    nc.vector.tensor_tensor_scan(
                        kvt[64:128, :], pk[64:128, 2 * T:3 * T], pk[64:128, 3 * T:4 * T],
                        init_i, add, add)
                    kv_prev[h] = kvt
                    # unbinding products
                    p12 = wpool.tile([128, 2 * T], f16, tag="p12", name=f"p12_{t}_{h}")
                    nc.vector.tensor_mul(p12[:, 0:T], kvt, fq_s)
                    nc.vector.tensor_mul(p12[:, T:2 * T], kvt, fqs_s)
                    # inverse fft (accumulate the two halves)
                    ps_vals = psC.tile([128, T], f32, tag="C", name=f"psv_{t}_{h}")
                    nc.tensor.matmul(ps_vals, lhsT=a_sb[:, 0:128], rhs=p12[:, 0:T],
                                     start=True, stop=False)
                    nc.tensor.matmul(ps_vals, lhsT=a_sb[:, 128:256], rhs=p12[:, T:2 * T],
                                     start=False, stop=True)
                    vt = wpool.tile([128, T], f16, tag=f"vals{h}", name=f"vals_{t}_{h}")
                    nc.scalar.copy(vt, ps_vals)
                    vals_sb.append(vt)
                # output projection (partial over this core's heads)
                for od in range(D // 128):
                    ps_out = psC.tile([128, T], f32, tag="C", name=f"pso_{t}_{od}")
                    for h in range(HPC):
                        nc.tensor.matmul(ps_out,
                                         lhsT=wo_sb[h][:, od * 128:(od + 1) * 128],
                                         rhs=vals_sb[h],
                                         start=(h == 0), stop=(h == HPC - 1))
                    ot = wpool.tile([128, T], f16, tag="ot", name=f"ot_{t}_{od}")
                    nc.scalar.copy(ot, ps_out)
                    nc.sync.dma_start(out=outT[od * 128:(od + 1) * 128, ts], in_=ot)
    nc.compile()
    return nc


def _make_in_maps(x, w_qkv, w_out):
    gmat, amat = _build_consts()
    x16 = x.astype(np.float16)
    wq16 = w_qkv.astype(np.float16)
    wo16 = (w_out * (SV / SO)).astype(np.float16)
    in_maps = []
    for c in range(NCORES):
        b, g = divmod(c, 2)
        heads = range(4 * g, 4 * g + 4)
        wq_cols = np.concatenate(
            [wq16[:, comp * D + h * 128: comp * D + (h + 1) * 128]
             for h in heads for comp in range(3)], axis=1)
        wo_rows = np.concatenate(
            [wo16[h * 128:(h + 1) * 128, :] for h in heads], axis=0)
        in_maps.append({
            "xT": np.ascontiguousarray(x16[b].T),
            "wq": np.ascontiguousarray(wq_cols),
            "wo": np.ascontiguousarray(wo_rows),
            "gmat": gmat,
            "amat": amat,
        })
    return in_maps


_NC_CACHE = None


def _get_program():
    global _NC_CACHE
    if _NC_CACHE is None:
        _NC_CACHE = _build_program()
    return _NC_CACHE


def kernel(x, w_qkv, w_out, _trace=False, _results_out=None):
    import sys
    if "/opt/trn_rl_repo" not in sys.path:
        sys.path.insert(0, "/opt/trn_rl_repo")
    from concourse.bass_utils import run_bass_kernel_spmd

    x = np.asarray(x)
    w_qkv = np.asarray(w_qkv)
    w_out = np.asarray(w_out)
    nc = _get_program()
    in_maps = _make_in_maps(x, w_qkv, w_out)
    res = run_bass_kernel_spmd(nc, in_maps, list(range(NCORES)), trace=_trace)
    if _results_out is not None:
        _results_out.append(res)
    out = np.empty((B, S, D), np.float32)
    for b in range(B):
        p0 = res.results[2 * b]["outT"].astype(np.float32)
        p1 = res.results[2 * b + 1]["outT"].astype(np.float32)
        out[b] = (p0 + p1).T * SO
    return out



# revision 3
# speedup vs baseline: 1.6172x; 1.6172x over previous
"""HRR self-attention (causal holographic binding) on 8 Trainium2 cores.

Math (per batch b, head h, reference semantics):
    qkv = x @ w_qkv ; q,k,v heads of HD=128
    fq,fk,fv = fft(q|k|v, axis=-1)          (length-128 FFT == matmul with DFT matrix)
    kv   = cumsum(fk*fv, axis=seq)          (causal binding)
    vals = ifft(kv * conj(fq)).real
    out  = vals @ w_out

Implementation notes:
  * FFT/iFFT are 128x128 matmuls (HD == 128 == PE tile).  Real-input FFT is
    conjugate-symmetric; the packings below place the spectra so the causal
    cumsum is ONE full-height tensor_tensor_scan and the binding products are
    TWO full-height element-wise muls:
      R_k = GR^T k  : [Re 0..63 | ReNyq | Re 1..63]
      I_k = GI^T k  : [Im 0..63 |   0   | Im 1..63]
      M1  = Gk^T v  : [Re 0..63 | ReNyq | Im 1..63]
      M2  = GM2^T v : [Im 0..63 |   0   | -Re 1..63]
      scan state = (R_k*M1 + state) - (I_k*M2)   per token
        rows 0..63 : cumsum(ReK ReV - ImK ImV)   = Re(kv)
        row  64    : cumsum(NyqK NyqV)           = Nyq(kv)
        rows 65..  : cumsum(ReK ImV + ImK ReV)   = Im(kv)
    Unbinding (fq = Gk^T q, fqs = Gs0^T q, inverse via A1|A2) as before.
  * Sharding: core c = 2*b + g handles batch b, heads 4g..4g+3.  Each core
    emits a partial out^T; the host sums the pair of partials per batch.
  * Emission is software-pipelined per head-slot s (= 4*chunk + head):
    proj(s) matmuls interleave with spectra(s-1) matmuls + DVE bind/scan,
    ifft(s-2), and the chunk output projection trails two slots.  PSUM is
    allocated as 8 single-bank tiles (proj 3 + spectra 3 + ifft/out 2) so the
    PE never waits on PSUM->SBUF drains.
  * All matmuls fp16 (fp32 PSUM).  DFT matrices pre-scaled by 1/16; host
    undoes the net scale.
"""

import numpy as np

B, S, D, H = 4, 4096, 1024, 8
HD = 128
NCORES = 8
HPC = H // 2            # heads per core
T = 512                 # token chunk (PSUM bank = 512 fp32)
NT = S // T
KK = D // 128           # contraction tiles for the qkv projection
NSLOT = NT * HPC        # 32 head-slots
FS = 16.0               # scale folded into each forward DFT matrix
SV = 16.0               # vals stored as vals/SV
SO = 16.0               # outT stored as out/SO  (host multiplies back)


def _build_consts():
    """Forward packed DFT matrices [Gk|GR|GI|GM2|Gs0] and inverse [A1|A2].

    Column j of each forward matrix produces packed row j (out = G^T x):
      Gk : [cos | nyq | -sin]   -> [Re | ReNyq | Im]   (fq, M1)
      GR : [cos | nyq |  cos]   -> [Re | ReNyq | Re]   (R_k)
      GI : [-sin |  0 | -sin]   -> [Im |   0   | Im]   (I_k)
      GM2: [-sin |  0 | -cos]   -> [Im |   0   | -Re]  (M2)
      Gs0: [-sin |  0 |  cos]   -> [Im |   0   | Re]   (fqs)
    """
    n = HD
    a = np.arange(n)
    cos_aj = np.cos(2 * np.pi * np.outer(a, np.arange(64)) / n)   # [a, j]
    sin_aj = np.sin(2 * np.pi * np.outer(a, np.arange(64)) / n)
    nyq = np.where(a % 2 == 0, 1.0, -1.0)              # (-1)^a

    def fwd(re_cols, col64, im_cols):
        M = np.zeros((n, n))
        M[:, :64] = re_cols
        M[:, 64] = col64
        M[:, 65:] = im_cols[:, 1:]
        return M

    Gk = fwd(cos_aj, nyq, -sin_aj)
    GR = fwd(cos_aj, nyq, cos_aj)
    GI = fwd(-sin_aj, 0.0, -sin_aj)
    GM2 = fwd(-sin_aj, 0.0, -cos_aj)
    Gs0 = fwd(-sin_aj, 0.0, cos_aj)

    # inverse: vals_n = sum_p A1[p,n] P1[p] + A2[p,n] P2[p]
    cos_jn = np.cos(2 * np.pi * np.outer(np.arange(64), a) / n)   # [j, n]
    sin_jn = np.sin(2 * np.pi * np.outer(np.arange(64), a) / n)
    w = np.full(64, 2.0)
    w[0] = 1.0
    A1 = np.zeros((n, n))
    A1[:64, :] = w[:, None] * cos_jn / n
    A1[64, :] = np.where(np.arange(n) % 2 == 0, 1.0, -1.0) / n    # Nyquist (-1)^n
    A1[65:, :] = 2.0 * cos_jn[1:] / n
    A2 = np.zeros((n, n))
    A2[:64, :] = 2.0 * sin_jn / n
    A2[64, :] = 0.0
    A2[65:, :] = -2.0 * sin_jn[1:] / n

    Amul = FS ** 3 / SV
    gmat = np.concatenate(
        [Gk / FS, GR / FS, GI / FS, GM2 / FS, Gs0 / FS], axis=1
    ).astype(np.float16)                                          # [128, 640]
    amat = np.concatenate([A1 * Amul, A2 * Amul], axis=1).astype(np.float16)
    return gmat, amat


def _build_program():
    import concourse.bass as bass
    import concourse.bacc as bacc
    import concourse.mybir as mybir
    import concourse.tile as tile

    f16 = mybir.dt.float16
    f32 = mybir.dt.float32
    add = mybir.AluOpType.add
    sub = mybir.AluOpType.subtract

    nc = bacc.Bacc("TRN2", target_bir_lowering=False, debug=False)
    xT = nc.dram_tensor("xT", [D, S], f16, kind="ExternalInput").ap()
    wq = nc.dram_tensor("wq", [D, 3 * HPC * 128], f16, kind="ExternalInput").ap()
    wo = nc.dram_tensor("wo", [HPC * 128, D], f16, kind="ExternalInput").ap()
    gmat = nc.dram_tensor("gmat", [128, 640], f16, kind="ExternalInput").ap()
    amat = nc.dram_tensor("amat", [128, 256], f16, kind="ExternalInput").ap()
    outT = nc.dram_tensor("outT", [D, S], f16, kind="ExternalOutput").ap()

    # gmat column offsets
    GK, GRC, GIC, GM2C, GS0 = 0, 128, 256, 384, 512

    with tile.TileContext(nc) as tc:
        with (
            tc.tile_pool(name="consts", bufs=1) as cpool,
            tc.tile_pool(name="xin", bufs=2) as xpool,
            tc.tile_pool(name="qkvp", bufs=2) as qkvpool,
            tc.tile_pool(name="spectp", bufs=2) as spool,
            tc.tile_pool(name="pkp", bufs=2) as pkpool,
            tc.tile_pool(name="kvp", bufs=2) as kvpool,
            tc.tile_pool(name="p12p", bufs=2) as p12pool,
            tc.tile_pool(name="valp", bufs=2) as vpool,
            tc.tile_pool(name="otp", bufs=3) as otpool,
            tc.tile_pool(name="psP", bufs=3, space="PSUM") as psP,
            tc.tile_pool(name="psS", bufs=3, space="PSUM") as psS,
            tc.tile_pool(name="psX", bufs=2, space="PSUM") as psX,
        ):
            wq_sb = []
            for k in range(KK):
                wqt = cpool.tile([128, 3 * HPC * 128], f16, name=f"wq{k}")
                nc.sync.dma_start(out=wqt, in_=wq[k * 128:(k + 1) * 128, :])
                wq_sb.append(wqt)
            wo_sb = []
            for h in range(HPC):
                wot = cpool.tile([128, D], f16, name=f"wo{h}")
                nc.sync.dma_start(out=wot, in_=wo[h * 128:(h + 1) * 128, :])
                wo_sb.append(wot)
            g_sb = cpool.tile([128, 640], f16, name="g_sb")
            nc.sync.dma_start(out=g_sb, in_=gmat)
            a_sb = cpool.tile([128, 256], f16, name="a_sb")
            nc.sync.dma_start(out=a_sb, in_=amat)

            xk_tiles = {}      # (t, k) -> tile
            qkv_sb = {}        # s -> [128, 3T] (q|k|v)
            spect_sb = {}      # s -> [128, 6T] (R|I|M1|M2|fq|fqs)
            kv_cur = {}        # h -> latest kv tile
            p12_sb = {}        # s -> [128, 2T]
            vals_sb = {}       # s -> [128, T]

            def emit_xdma(t):
                for k in range(KK):
                    xt = xpool.tile([128, T], f16, tag=f"xk{k}", name=f"x_{t}_{k}")
                    nc.sync.dma_start(out=xt, in_=xT[k * 128:(k + 1) * 128,
                                                    t * T:(t + 1) * T])
                    xk_tiles[(t, k)] = xt

            def emit_proj_comp(s, comp, ps_tile):
                t, h = divmod(s, HPC)
                col0 = (h * 3 + comp) * 128
                for k in range(KK):
                    nc.tensor.matmul(
                        ps_tile,
                        lhsT=wq_sb[k][:, col0:col0 + 128],
                        rhs=xk_tiles[(t, k)],
                        start=(k == 0),
                        stop=(k == KK - 1),
                    )

            def emit_spect_mm(s, gcol, rhs_slice):
                pst = psS.tile([128, T], f32, tag="S", name=f"psS_{s}_{gcol}")
                nc.tensor.matmul(pst, lhsT=g_sb[:, gcol:gcol + 128],
                                 rhs=qkv_sb[s][:, rhs_slice])
                return pst

            emit_xdma(0)
            for s in range(NSLOT + 3):
                t, h = divmod(s, HPC)
                if s % HPC == 2 and t + 1 < NT:
                    emit_xdma(t + 1)
                cur = s < NSLOT
                prv = s - 1 if 1 <= s <= NSLOT else None
                pv2 = s - 2 if 2 <= s <= NSLOT + 1 else None

                if cur:
                    qkv_sb[s] = qkvpool.tile([128, 3 * T], f16, tag="qkv",
                                             name=f"qkv_{s}")
                if prv is not None:
                    spect_sb[prv] = spool.tile([128, 6 * T], f16, tag="spect",
                                               name=f"spect_{prv}")

                # --- proj q | spectra R_k, I_k of prev slot ---
                if cur:
                    psq = psP.tile([128, T], f32, tag="P", name=f"psq_{s}")
                    emit_proj_comp(s, 0, psq)
                    nc.scalar.copy(qkv_sb[s][:, 0:T], psq)
                if prv is not None:
                    ks = qkv_sb[prv][:, T:2 * T]
                    psr = emit_spect_mm(prv, GRC, slice(T, 2 * T))
                    psi = emit_spect_mm(prv, GIC, slice(T, 2 * T))
                    nc.vector.tensor_copy(spect_sb[prv][:, 0:T], psr)
                    nc.vector.tensor_copy(spect_sb[prv][:, T:2 * T], psi)

                # --- proj k | spectra M1, M2 ---
                if cur:
                    psk = psP.tile([128, T], f32, tag="P", name=f"psk_{s}")
                    emit_proj_comp(s, 1, psk)
                    nc.scalar.copy(qkv_sb[s][:, T:2 * T], psk)
                if prv is not None:
                    psm1 = emit_spect_mm(prv, GK, slice(2 * T, 3 * T))
                    psm2 = emit_spect_mm(prv, GM2C, slice(2 * T, 3 * T))
                    nc.scalar.copy(spect_sb[prv][:, 2 * T:3 * T], psm1)
                    nc.scalar.copy(spect_sb[prv][:, 3 * T:4 * T], psm2)

                # --- proj v | spectra fq, fqs + bind + scan + unbind muls ---
                if cur:
                    psv = psP.tile([128, T], f32, tag="P", name=f"psv_{s}")
                    emit_proj_comp(s, 2, psv)
                    nc.scalar.copy(qkv_sb[s][:, 2 * T:3 * T], psv)
                if prv is not None:
                    pt, ph = divmod(prv, HPC)
                    psfq = emit_spect_mm(prv, GK, slice(0, T))
                    psfqs = emit_spect_mm(prv, GS0, slice(0, T))
                    nc.vector.tensor_copy(spect_sb[prv][:, 4 * T:5 * T], psfq)
                    nc.vector.tensor_copy(spect_sb[prv][:, 5 * T:6 * T], psfqs)
                    sp = spect_sb[prv]
                    pk = pkpool.tile([128, 2 * T], f16, tag="pk", name=f"pk_{prv}")
                    nc.vector.tensor_mul(pk[:, 0:T], sp[:, 0:T], sp[:, 2 * T:3 * T])
                    nc.vector.tensor_mul(pk[:, T:2 * T], sp[:, T:2 * T],
                                         sp[:, 3 * T:4 * T])
                    kvt = kvpool.tile([128, T], f16, tag=f"kv{ph}",
                                      name=f"kv_{prv}")
                    init = 0.0 if pt == 0 else kv_cur[ph][:, T - 1:T]
                    nc.vector.tensor_tensor_scan(
                        kvt, pk[:, 0:T], pk[:, T:2 * T], init, add, sub)
                    kv_cur[ph] = kvt
                    p12 = p12pool.tile([128, 2 * T], f16, tag="p12",
                                       name=f"p12_{prv}")
                    nc.vector.tensor_mul(p12[:, 0:T], kvt, sp[:, 4 * T:5 * T])
                    nc.vector.tensor_mul(p12[:, T:2 * T], kvt, sp[:, 5 * T:6 * T])
                    p12_sb[prv] = p12

                # --- ifft of slot s-2 ---
                if pv2 is not None:
                    p12 = p12_sb.pop(pv2)
                    v2t, v2h = divmod(pv2, HPC)
                    psval = psX.tile([128, T], f32, tag="X", name=f"psval_{pv2}")
                    nc.tensor.matmul(psval, lhsT=a_sb[:, 0:128], rhs=p12[:, 0:T],
                                     start=True, stop=False)
                    nc.tensor.matmul(psval, lhsT=a_sb[:, 128:256],
                                     rhs=p12[:, T:2 * T], start=False, stop=True)
                    vt = vpool.tile([128, T], f16, tag=f"v{v2h}", name=f"vals_{pv2}")
                    nc.scalar.copy(vt, psval)
                    vals_sb[pv2] = vt

                # --- output projection for chunk t-1 (all its vals are ready) ---
                if s % HPC == 2 and 1 <= t <= NT:
                    ot_chunk = t - 1
                    ts_ = slice(ot_chunk * T, (ot_chunk + 1) * T)
                    for od in range(D // 128):
                        ps_out = psX.tile([128, T], f32, tag="X",
                                          name=f"pso_{ot_chunk}_{od}")
                        for hh in range(HPC):
                            nc.tensor.matmul(
                                ps_out,
                                lhsT=wo_sb[hh][:, od * 128:(od + 1) * 128],
                                rhs=vals_sb[ot_chunk * HPC + hh],
                                start=(hh == 0),
                                stop=(hh == HPC - 1),
                            )
                        ott = otpool.tile([128, T], f16, tag="ot",
                                          name=f"ot_{ot_chunk}_{od}")
                        nc.scalar.copy(ott, ps_out)
                        nc.sync.dma_start(out=outT[od * 128:(od + 1) * 128, ts_],
                                          in_=ott)
                    for hh in range(HPC):
                        del vals_sb[ot_chunk * HPC + hh]
    nc.compile()
    return nc


def _make_in_maps(x, w_qkv, w_out):
    gmat, amat = _build_consts()
    x16 = x.astype(np.float16)
    wq16 = w_qkv.astype(np.float16)
    wo16 = (w_out * (SV / SO)).astype(np.float16)
    in_maps = []
    for c in range(NCORES):
        b, g = divmod(c, 2)
        heads = range(4 * g, 4 * g + 4)
        wq_cols = np.concatenate(
            [wq16[:, comp * D + h * 128: comp * D + (h + 1) * 128]
             for h in heads for comp in range(3)], axis=1)
        wo_rows = np.concatenate(
            [wo16[h * 128:(h + 1) * 128, :] for h in heads], axis=0)
        in_maps.append({
            "xT": np.ascontiguousarray(x16[b].T),
            "wq": np.ascontiguousarray(wq_cols),
            "wo": np.ascontiguousarray(wo_rows),
            "gmat": gmat,
            "amat": amat,
        })
    return in_maps


_NC_CACHE = None


def _get_program():
    global _NC_CACHE
    if _NC_CACHE is None:
        _NC_CACHE = _build_program()
    return _NC_CACHE


def kernel(x, w_qkv, w_out, _trace=False, _results_out=None):
    import sys
    if "/opt/trn_rl_repo" not in sys.path:
        sys.path.insert(0, "/opt/trn_rl_repo")
    from concourse.bass_utils import run_bass_kernel_spmd

    x = np.asarray(x)
    w_qkv = np.asarray(w_qkv)
    w_out = np.asarray(w_out)
    nc = _get_program()
    in_maps = _make_in_maps(x, w_qkv, w_out)
    res = run_bass_kernel_spmd(nc, in_maps, list(range(NCORES)), trace=_trace)
    if _results_out is not None:
        _results_out.append(res)
    out = np.empty((B, S, D), np.float32)
    for b in range(B):
        p0 = res.results[2 * b]["outT"].astype(np.float32)
        p1 = res.results[2 * b + 1]["outT"].astype(np.float32)
        out[b] = (p0 + p1).T * SO
    return out
